# revision 7
# baseline (speedup 1.0000x reference)
"""Sparse-attention Bass kernel for Trainium2 (8 NeuronCores).

Problem (per batch element b of 8):
    scores = (q @ k^T) * scale            [2048, 2048]
    scores = where(mask[k], -1e9, scores)
    scores = scores * ratio[b]
    attn   = softmax(scores, axis=-1)
    out    = attn @ v                      [2048, 512]

Sharding: batch dim (8) -> one NeuronCore each (SPMD, same NEFF).

Device layout ("S^T layout"): scores are computed transposed,
S^T[k, q] = K @ Q^T (keys on partitions, queries on the free dim), so
  - the key-mask bias is a per-partition bias -> fused into the exp
    activation on the Scalar engine for free,
  - the AV matmul (contraction over keys) needs no transposes:
    lhsT = P^T tile [128k, 128q] (stationary), rhs = V [128k, 512d],
  - softmax denominators (sum over keys = partitions) come from a
    ones-vector matmul: rowsum[1, q] += ones[128,1].T @ P^T[128, q].

Normalization (divide by rowsum) is done on the host: the device returns
the unnormalized O = exp(S) @ V plus the row sums.
The scale*ratio[b] factor is folded into q on the host.

Written in raw Bass (explicit engine programs + semaphores): the walrus
build in this container allows at most ONE semaphore wait per
instruction, which the Tile scheduler's auto-generated waits violate.
Standalone wait_ge instructions sidestep the limit.

Engine roles:
  sync   (SP) : input DMAs (one HWDGE ring, FIFO -> one dma_sem)
  tensor (PE) : QK^T matmuls, rowsum matmuls, AV matmuls (float32r)
  scalar (ACT): exp (+mask bias), PSUM->SBUF copies, output DMAs
                (on ACT's own HWDGE ring so they don't queue behind
                the input DMAs)
"""

import sys

for _p in ("/opt/trn_rl_repo", "/opt/pypackages"):
    if _p not in sys.path:
        sys.path.append(_p)

import numpy as np
from contextlib import ExitStack

import concourse.bass as bass
from concourse import mybir
from concourse.bass_utils import run_bass_kernel_spmd

B, LQ, LK, D = 8, 2048, 2048, 512
P = 128
NCORES = 8
F32 = mybir.dt.float32
F32R = mybir.dt.float32r
NEG = np.float32(-1e9)

DT = D // P        # 4 d-tiles (contraction for QK^T)
KT = LK // P       # 16 key tiles (partitions of S^T)
QBS = 512          # queries per PSUM block (free dim of S^T)
QB = LQ // QBS     # 4 query superblocks
QTPB = QBS // P    # 4 query tiles (of 128) per superblock

# kq packing: [128, 16384] =
#   A(cols 0:2048)      kT d-tiles, keys 0:512
#   B(cols 2048:4096)   qT d-tiles, queries 0:512
#   C(cols 4096:10240)  kT d-tiles, keys 512:2048
#   D(cols 10240:16384) qT d-tiles, queries 512:2048
KQ_COLS = 4 * (LK + LQ) // P * P  # 16384
C0, D0 = 4096, 10240


def _kcol(d, j):
    """column of kq holding kT[d*128+p, j]"""
    return d * 512 + j if j < 512 else C0 + d * 1536 + (j - 512)


def _qcol(d, i):
    return 2048 + d * 512 + i if i < 512 else D0 + d * 1536 + (i - 512)


def _build_bass():
    nc = bass.Bass()

    consts = nc.dram_tensor("consts", [P, KT], F32, kind="ExternalInput")
    onesd = nc.dram_tensor("onesd", [P, 1], F32R, kind="ExternalInput")
    kq = nc.dram_tensor("kq", [P, KQ_COLS], F32R, kind="ExternalInput")
    vv = nc.dram_tensor("vv", [P, KT * D], F32R, kind="ExternalInput")
    out_u = nc.dram_tensor("out_u", [LQ, D], F32, kind="ExternalOutput")
    sums = nc.dram_tensor("sums", [QB, QBS], F32, kind="ExternalOutput")

    EXP = mybir.ActivationFunctionType.Exp

    with ExitStack() as ctx:
        e = ctx.enter_context

        # SBUF
        sb_consts = e(nc.sbuf_tensor("sb_consts", [P, KT], F32))
        sb_ones = e(nc.sbuf_tensor("sb_ones", [P, 1], F32R))
        sb_kq = e(nc.sbuf_tensor("sb_kq", [P, KQ_COLS], F32R))
        sb_v = e(nc.sbuf_tensor("sb_v", [P, KT * D], F32R))
        # exp(S^T) tiles: [128k, 512q] per (qb parity, key tile)
        sb_pt = [
            [e(nc.sbuf_tensor(f"sb_pt{par}_{k}", [P, QBS], F32R)) for k in range(KT)]
            for par in range(2)
        ]
        sb_osb = [e(nc.sbuf_tensor(f"sb_osb{qt}", [P, D], F32)) for qt in range(QTPB)]
        sb_rs = [e(nc.sbuf_tensor(f"sb_rs{par}", [1, QBS], F32)) for par in range(2)]

        # PSUM: 7 of 8 banks
        ps = [e(nc.psum_tensor(f"ps{i}", [P, QBS], F32)) for i in range(3)]
        po = [e(nc.psum_tensor(f"po{i}", [P, D], F32)) for i in range(2)]
        rs = [e(nc.psum_tensor(f"rs{i}", [P, QBS], F32)) for i in range(2)]

        # one semaphore per input DMA: HWDGE DMAs on one ring may
        # complete out of order, so a shared counter can't identify which
        # transfer landed
        s_consts = e(nc.semaphore("s_consts"))
        s_ones = e(nc.semaphore("s_ones"))
        s_ab = e(nc.semaphore("s_ab"))
        s_c = e(nc.semaphore("s_c"))
        s_d = e(nc.semaphore("s_d"))
        s_v = e(nc.semaphore("s_v"))
        # per-output-buffer DMA-completion semaphores (buffer reuse gates)
        s_osb = [e(nc.semaphore(f"s_osb{qt}")) for qt in range(QTPB)]
        s_rsb = [e(nc.semaphore(f"s_rsb{par}")) for par in range(2)]
        pe_sem = e(nc.semaphore("pe_sem"))
        act_sem = e(nc.semaphore("act_sem"))

        # ---- semaphore tick bookkeeping ----
        # pe_sem increments: per qb: 16 QK-group finals, 1 rowsum final,
        # 4 AV finals = 21
        def tick_qk(qb, k):
            return qb * 21 + k + 1

        def tick_rs(qb):
            return qb * 21 + 17

        def tick_av(qb, qt):
            return qb * 21 + 18 + qt

        # act_sem increments: per qb: 16 exps, 1 rs copy, 4 po copies = 21
        def tick_exp(qb, k):
            return qb * 21 + k + 1

        def tick_rsc(qb):
            return qb * 21 + 17

        def tick_poc(qb, qt):
            return qb * 21 + 18 + qt

        with nc.Block() as block:

            @block.sync
            def _(sync):
                sync.dma_start(sb_kq[:, 0:C0], kq[:, 0:C0]).then_inc(s_ab, 16)
                sync.dma_start(sb_consts[:, :], consts[:, :]).then_inc(s_consts, 16)
                sync.dma_start(sb_ones[:, :], onesd[:, :]).then_inc(s_ones, 16)
                sync.dma_start(sb_kq[:, C0:D0], kq[:, C0:D0]).then_inc(s_c, 16)
                sync.dma_start(
                    sb_kq[:, D0:KQ_COLS], kq[:, D0:KQ_COLS]
                ).then_inc(s_d, 16)
                sync.dma_start(sb_v[:, :], vv[:, :]).then_inc(s_v, 16)

            @block.tensor
            def _(tensor):
                last_wait = {}  # sem name -> value already waited for

                def wait(sem, val, name):
                    if val > last_wait.get(name, -1):
                        tensor.wait_ge(sem, val)
                        last_wait[name] = val

                for qb in range(QB):
                    # ---- QK^T + rowsum phase ----
                    for k in range(KT):
                        g = qb * KT + k  # global k-iteration index
                        # input availability
                        if k < 4 and qb == 0:
                            wait(s_ab, 16, "ab")
                            wait(s_ones, 16, "ones")
                        elif qb == 0:
                            wait(s_c, 16, "c")
                        else:
                            wait(s_d, 16, "d")
                        # ps[g%3] must have been consumed by exp of g-3
                        if g >= 3:
                            g3 = g - 3
                            wait(act_sem, tick_exp(g3 // KT, g3 % KT), "act")
                        for d in range(DT):
                            mm = tensor.matmul(
                                ps[g % 3][:, :],
                                lhsT=sb_kq[:, _kcol(d, k * P):_kcol(d, k * P) + P],
                                rhs=sb_kq[:, _qcol(d, qb * QBS):_qcol(d, qb * QBS) + QBS],
                                start=(d == 0),
                                stop=(d == DT - 1),
                            )
                            if d == DT - 1:
                                mm.then_inc(pe_sem, 1)
                        # rowsum for k-2 (give ACT pipeline slack)
                        if k >= 2:
                            j = k - 2
                            wait(act_sem, tick_exp(qb, j), "act")
                            mm = tensor.matmul(
                                rs[qb % 2][0:1, :],
                                lhsT=sb_ones[:, :],
                                rhs=sb_pt[qb % 2][j][:, :],
                                start=(j == 0),
                                stop=(j == KT - 1),
                            )
                            if j == KT - 1:
                                mm.then_inc(pe_sem, 1)
                    for j in (KT - 2, KT - 1):
                        wait(act_sem, tick_exp(qb, j), "act")
                        mm = tensor.matmul(
                            rs[qb % 2][0:1, :],
                            lhsT=sb_ones[:, :],
                            rhs=sb_pt[qb % 2][j][:, :],
                            start=(j == 0),
                            stop=(j == KT - 1),
                        )
                        if j == KT - 1:
                            mm.then_inc(pe_sem, 1)

                    # ---- AV phase ----
                    wait(s_v, 16, "v")
                    for qt in range(QTPB):
                        # po[qt%2] consumed by copy of (qb,qt-2) / (qb-1,qt+2)
                        if qt >= 2:
                            wait(act_sem, tick_poc(qb, qt - 2), "act")
                        elif qb >= 1:
                            wait(act_sem, tick_poc(qb - 1, qt + 2), "act")
                        for k in range(KT):
                            mm = tensor.matmul(
                                po[qt % 2][:, :],
                                lhsT=sb_pt[qb % 2][k][:, qt * P:(qt + 1) * P],
                                rhs=sb_v[:, k * D:(k + 1) * D],
                                start=(k == 0),
                                stop=(k == KT - 1),
                            )
                            if k == KT - 1:
                                mm.then_inc(pe_sem, 1)

            @block.scalar
            def _(scalar):
                last_wait = {}

                def wait(sem, val, name):
                    if val > last_wait.get(name, -1):
                        scalar.wait_ge(sem, val)
                        last_wait[name] = val

                wait(s_consts, 16, "consts")
                for qb in range(QB):
                    for k in range(KT):
                        g = qb * KT + k
                        wait(pe_sem, tick_qk(qb, k), "pe")
                        scalar.activation(
                            sb_pt[qb % 2][k][:, :],
                            ps[g % 3][:, :],
                            EXP,
                            bias=sb_consts[:, k:k + 1],
                            scale=1.0,
                        ).then_inc(act_sem, 1)
                    # rowsum copy + DMA (ACT's own HWDGE ring)
                    if qb >= 2:
                        wait(s_rsb[qb % 2], 16 * (qb // 2), f"rsb{qb % 2}")
                    wait(pe_sem, tick_rs(qb), "pe")
                    scalar.copy(sb_rs[qb % 2][:, :], rs[qb % 2][0:1, :]).then_inc(
                        act_sem, 1
                    )
                    # self-wait: the DMA engine reads sb_rs asynchronously,
                    # so the copy must have fully drained first
                    wait(act_sem, tick_rsc(qb), "act")
                    scalar.dma_start(
                        sums[qb:qb + 1, :], sb_rs[qb % 2][:, :]
                    ).then_inc(s_rsb[qb % 2], 16)
                    # AV output copies + DMAs
                    for qt in range(QTPB):
                        if qb >= 1:
                            wait(s_osb[qt], 16 * qb, f"osb{qt}")
                        wait(pe_sem, tick_av(qb, qt), "pe")
                        scalar.copy(sb_osb[qt][:, :], po[qt % 2][:, :]).then_inc(
                            act_sem, 1
                        )
                        wait(act_sem, tick_poc(qb, qt), "act")
                        row = (qb * QTPB + qt) * P
                        scalar.dma_start(
                            out_u[row:row + P, :], sb_osb[qt][:, :]
                        ).then_inc(s_osb[qt], 16)
                # drain: all output DMAs landed
                for qt in range(QTPB):
                    scalar.wait_ge(s_osb[qt], 16 * QB)
                for par in range(2):
                    scalar.wait_ge(s_rsb[par], 16 * 2)

    return nc


_NC_CACHE = {}


def _get_nc():
    if "nc" not in _NC_CACHE:
        _NC_CACHE["nc"] = _build_bass()
    return _NC_CACHE["nc"]


def _pack_inputs(q, k, v, ratio, scale, attn_mask):
    """Host-side packing into the per-core flat layouts."""
    mult = np.float32(scale) * ratio.astype(np.float32)  # [B]
    qs = q.astype(np.float32) * mult[:, None, None]      # [B, LQ, D]

    # kd[b, d, p, j] = k[b, j, d*128+p]
    kd = np.ascontiguousarray(k.astype(np.float32).transpose(0, 2, 1)).reshape(
        B, DT, P, LK
    )
    qd = np.ascontiguousarray(qs.transpose(0, 2, 1)).reshape(B, DT, P, LQ)

    def reg(x, lo, hi):  # [B, DT, P, hi-lo] -> [B, P, DT*(hi-lo)]
        r = x[:, :, :, lo:hi].transpose(0, 2, 1, 3)
        return np.ascontiguousarray(r).reshape(B, P, -1)

    kq = np.concatenate(
        [reg(kd, 0, 512), reg(qd, 0, 512), reg(kd, 512, LK), reg(qd, 512, LQ)],
        axis=2,
    )  # [B, 128, 16384]

    vvl = (
        v.astype(np.float32).reshape(B, KT, P, D).transpose(0, 2, 1, 3)
    )
    vvl = np.ascontiguousarray(vvl).reshape(B, P, KT * D)

    # consts[b, p, k] = mask bias for key k*128+p
    bias = np.where(attn_mask, NEG, np.float32(0.0)).astype(np.float32)
    consts = np.ascontiguousarray(bias.reshape(B, KT, P).transpose(0, 2, 1))

    ones = np.ones((P, 1), dtype=np.float32)
    return kq, vvl, consts, ones


def kernel(q, k, v, ratio, scale, attn_mask, _want_trace=False):
    """Full inputs in, full output out. Shards batch across 8 cores."""
    assert q.shape == (B, LQ, D) and k.shape == (B, LK, D)
    nc = _get_nc()
    kq, vvl, consts, ones = _pack_inputs(q, k, v, ratio, scale, attn_mask)

    in_maps = [
        {"kq": kq[b], "vv": vvl[b], "consts": consts[b], "onesd": ones}
        for b in range(B)
    ]
    res = run_bass_kernel_spmd(
        nc, in_maps, core_ids=list(range(NCORES)), trace=_want_trace,
    )
    out_un = np.stack([r["out_u"] for r in res.results])         # [B, LQ, D]
    ssum = np.stack([r["sums"] for r in res.results]).reshape(B, LQ)
    out = out_un / ssum[:, :, None]
    if _want_trace:
        return out.astype(np.float32), res
    return out.astype(np.float32)


# revision 8
# speedup vs baseline: 258.9170x; 258.9170x over previous
"""Sparse-attention Bass kernel for Trainium2 (8 NeuronCores).

Problem (per batch element b of 8):
    scores = (q @ k^T) * scale            [2048, 2048]
    scores = where(mask[k], -1e9, scores)
    scores = scores * ratio[b]
    attn   = softmax(scores, axis=-1)
    out    = attn @ v                      [2048, 512]

Sharding: batch dim (8) -> one NeuronCore each (SPMD, same NEFF).

Device layout ("S^T layout"): scores are computed transposed,
S^T[k, q] = K @ Q^T (keys on partitions, queries on the free dim), so
  - the key-mask bias is a per-partition bias -> fused into the exp
    activation on the Scalar engine for free,
  - the AV matmul (contraction over keys) needs no transposes:
    lhsT = P^T tile [128k, 128q] (stationary), rhs = V [128k, 512d],
  - softmax denominators (sum over keys = partitions) come from a
    ones-vector matmul: rowsum[1, q] += ones[128,1].T @ P^T[128, q].

Normalization (divide by rowsum) is done on the host: the device returns
the unnormalized O = exp(S) @ V plus the row sums.
The scale*ratio[b] factor is folded into q on the host.

Written in raw Bass (explicit engine programs + semaphores): the walrus
build in this container allows at most ONE semaphore wait per
instruction, which the Tile scheduler's auto-generated waits violate.
Standalone wait_ge instructions sidestep the limit.

Engine roles:
  sync   (SP) : input DMAs (one HWDGE ring, FIFO -> one dma_sem)
  tensor (PE) : QK^T matmuls, rowsum matmuls, AV matmuls (float32r)
  scalar (ACT): exp (+mask bias), PSUM->SBUF copies, output DMAs
                (on ACT's own HWDGE ring so they don't queue behind
                the input DMAs)
"""

import sys

for _p in ("/opt/trn_rl_repo", "/opt/pypackages"):
    if _p not in sys.path:
        sys.path.append(_p)

import numpy as np
from contextlib import ExitStack

import concourse.bass as bass
from concourse import mybir
from concourse.bass_utils import run_bass_kernel_spmd

B, LQ, LK, D = 8, 2048, 2048, 512
P = 128
NCORES = 8
F32 = mybir.dt.float32
F32R = mybir.dt.float32r
NEG = np.float32(-1e9)

DT = D // P        # 4 d-tiles (contraction for QK^T)
KT = LK // P       # 16 key tiles (partitions of S^T)
QBS = 512          # queries per PSUM block (free dim of S^T)
QB = LQ // QBS     # 4 query superblocks
QTPB = QBS // P    # 4 query tiles (of 128) per superblock

# kq packing: [128, 16384] =
#   A(cols 0:2048)      kT d-tiles, keys 0:512
#   B(cols 2048:4096)   qT d-tiles, queries 0:512
#   C(cols 4096:10240)  kT d-tiles, keys 512:2048
#   D(cols 10240:16384) qT d-tiles, queries 512:2048
KQ_COLS = 4 * (LK + LQ) // P * P  # 16384
C0, D0 = 4096, 10240


def _kcol(d, j):
    """column of kq holding kT[d*128+p, j]"""
    return d * 512 + j if j < 512 else C0 + d * 1536 + (j - 512)


def _qcol(d, i):
    return 2048 + d * 512 + i if i < 512 else D0 + d * 1536 + (i - 512)


def _build_bass(niter=1):
    nc = bass.Bass()

    consts = nc.dram_tensor("consts", [P, KT], F32, kind="ExternalInput")
    onesd = nc.dram_tensor("onesd", [P, 1], F32R, kind="ExternalInput")
    kq = nc.dram_tensor("kq", [P, KQ_COLS], F32R, kind="ExternalInput")
    vv = nc.dram_tensor("vv", [P, KT * D], F32R, kind="ExternalInput")
    out_u = nc.dram_tensor("out_u", [LQ, D], F32, kind="ExternalOutput")
    sums = nc.dram_tensor("sums", [QB, QBS], F32, kind="ExternalOutput")

    EXP = mybir.ActivationFunctionType.Exp

    with ExitStack() as ctx:
        e = ctx.enter_context

        # SBUF
        sb_consts = e(nc.sbuf_tensor("sb_consts", [P, KT], F32))
        sb_ones = e(nc.sbuf_tensor("sb_ones", [P, 1], F32R))
        sb_kq = e(nc.sbuf_tensor("sb_kq", [P, KQ_COLS], F32R))
        sb_v = e(nc.sbuf_tensor("sb_v", [P, KT * D], F32R))
        # exp(S^T) tiles: [128k, 512q] per (qb parity, key tile)
        sb_pt = [
            [e(nc.sbuf_tensor(f"sb_pt{par}_{k}", [P, QBS], F32R)) for k in range(KT)]
            for par in range(2)
        ]
        sb_osb = [e(nc.sbuf_tensor(f"sb_osb{qt}", [P, D], F32)) for qt in range(QTPB)]
        sb_rs = [e(nc.sbuf_tensor(f"sb_rs{par}", [1, QBS], F32)) for par in range(2)]

        # PSUM: 7 of 8 banks
        ps = [e(nc.psum_tensor(f"ps{i}", [P, QBS], F32)) for i in range(3)]
        po = [e(nc.psum_tensor(f"po{i}", [P, D], F32)) for i in range(2)]
        rs = [e(nc.psum_tensor(f"rs{i}", [P, QBS], F32)) for i in range(2)]

        # one semaphore per input DMA: HWDGE DMAs on one ring may
        # complete out of order, so a shared counter can't identify which
        # transfer landed
        s_consts = e(nc.semaphore("s_consts"))
        s_ones = e(nc.semaphore("s_ones"))
        s_ab = e(nc.semaphore("s_ab"))
        s_c = e(nc.semaphore("s_c"))
        s_d = e(nc.semaphore("s_d"))
        s_v = e(nc.semaphore("s_v"))
        # per-output-buffer DMA-completion semaphores (buffer reuse gates)
        s_osb = [e(nc.semaphore(f"s_osb{qt}")) for qt in range(QTPB)]
        s_rsb = [e(nc.semaphore(f"s_rsb{par}")) for par in range(2)]
        pe_sem = e(nc.semaphore("pe_sem"))
        act_sem = e(nc.semaphore("act_sem"))

        # ---- semaphore tick bookkeeping ----
        # gb = global block index (niter * QB blocks total); data block
        # qb = gb % QB.
        # pe_sem increments: per gb: 16 QK-group finals, 1 rowsum final,
        # 4 AV finals = 21
        def tick_qk(gb, k):
            return gb * 21 + k + 1

        def tick_rs(gb):
            return gb * 21 + 17

        def tick_av(gb, qt):
            return gb * 21 + 18 + qt

        # act_sem increments: per gb: 16 exps, 1 rs copy, 4 po copies = 21
        def tick_exp(gb, k):
            return gb * 21 + k + 1

        def tick_rsc(gb):
            return gb * 21 + 17

        def tick_poc(gb, qt):
            return gb * 21 + 18 + qt

        with nc.Block() as block:

            @block.sync
            def _(sync):
                sync.dma_start(sb_kq[:, 0:C0], kq[:, 0:C0]).then_inc(s_ab, 16)
                sync.dma_start(sb_consts[:, :], consts[:, :]).then_inc(s_consts, 16)
                sync.dma_start(sb_ones[:, :], onesd[:, :]).then_inc(s_ones, 16)
                sync.dma_start(sb_kq[:, C0:D0], kq[:, C0:D0]).then_inc(s_c, 16)
                sync.dma_start(
                    sb_kq[:, D0:KQ_COLS], kq[:, D0:KQ_COLS]
                ).then_inc(s_d, 16)
                sync.dma_start(sb_v[:, :], vv[:, :]).then_inc(s_v, 16)

            @block.tensor
            def _(tensor):
                last_wait = {}  # sem name -> value already waited for

                def wait(sem, val, name):
                    if val > last_wait.get(name, -1):
                        tensor.wait_ge(sem, val)
                        last_wait[name] = val

                for gb in range(niter * QB):
                    qb = gb % QB
                    # ---- QK^T + rowsum phase ----
                    for k in range(KT):
                        g = gb * KT + k  # global k-iteration index
                        # input availability
                        if k < 4 and qb == 0:
                            wait(s_ab, 16, "ab")
                            wait(s_ones, 16, "ones")
                        elif qb == 0:
                            wait(s_c, 16, "c")
                        else:
                            wait(s_d, 16, "d")
                        # ps[g%3] must have been consumed by exp of g-3
                        if g >= 3:
                            g3 = g - 3
                            wait(act_sem, tick_exp(g3 // KT, g3 % KT), "act")
                        for d in range(DT):
                            mm = tensor.matmul(
                                ps[g % 3][:, :],
                                lhsT=sb_kq[:, _kcol(d, k * P):_kcol(d, k * P) + P],
                                rhs=sb_kq[:, _qcol(d, qb * QBS):_qcol(d, qb * QBS) + QBS],
                                start=(d == 0),
                                stop=(d == DT - 1),
                            )
                            if d == DT - 1:
                                mm.then_inc(pe_sem, 1)
                        # rowsum for k-2 (give ACT pipeline slack)
                        if k >= 2:
                            j = k - 2
                            wait(act_sem, tick_exp(gb, j), "act")
                            mm = tensor.matmul(
                                rs[gb % 2][0:1, :],
                                lhsT=sb_ones[:, :],
                                rhs=sb_pt[gb % 2][j][:, :],
                                start=(j == 0),
                                stop=(j == KT - 1),
                            )
                            if j == KT - 1:
                                mm.then_inc(pe_sem, 1)
                    for j in (KT - 2, KT - 1):
                        wait(act_sem, tick_exp(gb, j), "act")
                        mm = tensor.matmul(
                            rs[gb % 2][0:1, :],
                            lhsT=sb_ones[:, :],
                            rhs=sb_pt[gb % 2][j][:, :],
                            start=(j == 0),
                            stop=(j == KT - 1),
                        )
                        if j == KT - 1:
                            mm.then_inc(pe_sem, 1)

                    # ---- AV phase ----
                    wait(s_v, 16, "v")
                    for qt in range(QTPB):
                        # po[qt%2] consumed by copy of (gb,qt-2) / (gb-1,qt+2)
                        if qt >= 2:
                            wait(act_sem, tick_poc(gb, qt - 2), "act")
                        elif gb >= 1:
                            wait(act_sem, tick_poc(gb - 1, qt + 2), "act")
                        for k in range(KT):
                            mm = tensor.matmul(
                                po[qt % 2][:, :],
                                lhsT=sb_pt[gb % 2][k][:, qt * P:(qt + 1) * P],
                                rhs=sb_v[:, k * D:(k + 1) * D],
                                start=(k == 0),
                                stop=(k == KT - 1),
                            )
                            if k == KT - 1:
                                mm.then_inc(pe_sem, 1)

            @block.scalar
            def _(scalar):
                last_wait = {}

                def wait(sem, val, name):
                    if val > last_wait.get(name, -1):
                        scalar.wait_ge(sem, val)
                        last_wait[name] = val

                wait(s_consts, 16, "consts")
                for gb in range(niter * QB):
                    qb = gb % QB
                    for k in range(KT):
                        g = gb * KT + k
                        wait(pe_sem, tick_qk(gb, k), "pe")
                        scalar.activation(
                            sb_pt[gb % 2][k][:, :],
                            ps[g % 3][:, :],
                            EXP,
                            bias=sb_consts[:, k:k + 1],
                            scale=1.0,
                        ).then_inc(act_sem, 1)
                    # rowsum copy + DMA (ACT's own HWDGE ring)
                    if gb >= 2:
                        wait(s_rsb[gb % 2], 16 * (gb // 2), f"rsb{gb % 2}")
                    wait(pe_sem, tick_rs(gb), "pe")
                    scalar.copy(sb_rs[gb % 2][:, :], rs[gb % 2][0:1, :]).then_inc(
                        act_sem, 1
                    )
                    # self-wait: the DMA engine reads sb_rs asynchronously,
                    # so the copy must have fully drained first
                    wait(act_sem, tick_rsc(gb), "act")
                    scalar.dma_start(
                        sums[qb:qb + 1, :], sb_rs[gb % 2][:, :]
                    ).then_inc(s_rsb[gb % 2], 16)
                    # AV output copies + DMAs
                    for qt in range(QTPB):
                        if gb >= 1:
                            wait(s_osb[qt], 16 * gb, f"osb{qt}")
                        wait(pe_sem, tick_av(gb, qt), "pe")
                        scalar.copy(sb_osb[qt][:, :], po[qt % 2][:, :]).then_inc(
                            act_sem, 1
                        )
                        wait(act_sem, tick_poc(gb, qt), "act")
                        row = (qb * QTPB + qt) * P
                        scalar.dma_start(
                            out_u[row:row + P, :], sb_osb[qt][:, :]
                        ).then_inc(s_osb[qt], 16)
                # drain: all output DMAs landed
                for qt in range(QTPB):
                    scalar.wait_ge(s_osb[qt], 16 * QB * niter)
                for par in range(2):
                    scalar.wait_ge(s_rsb[par], 16 * 2 * niter)

    return nc


_NC_CACHE = {}


def _get_nc(niter=1):
    if niter not in _NC_CACHE:
        _NC_CACHE[niter] = _build_bass(niter)
    return _NC_CACHE[niter]


def _pack_inputs(q, k, v, ratio, scale, attn_mask):
    """Host-side packing into the per-core flat layouts."""
    mult = np.float32(scale) * ratio.astype(np.float32)  # [B]
    qs = q.astype(np.float32) * mult[:, None, None]      # [B, LQ, D]

    # kd[b, d, p, j] = k[b, j, d*128+p]
    kd = np.ascontiguousarray(k.astype(np.float32).transpose(0, 2, 1)).reshape(
        B, DT, P, LK
    )
    qd = np.ascontiguousarray(qs.transpose(0, 2, 1)).reshape(B, DT, P, LQ)

    def reg(x, lo, hi):  # [B, DT, P, hi-lo] -> [B, P, DT*(hi-lo)]
        r = x[:, :, :, lo:hi].transpose(0, 2, 1, 3)
        return np.ascontiguousarray(r).reshape(B, P, -1)

    kq = np.concatenate(
        [reg(kd, 0, 512), reg(qd, 0, 512), reg(kd, 512, LK), reg(qd, 512, LQ)],
        axis=2,
    )  # [B, 128, 16384]

    vvl = (
        v.astype(np.float32).reshape(B, KT, P, D).transpose(0, 2, 1, 3)
    )
    vvl = np.ascontiguousarray(vvl).reshape(B, P, KT * D)

    # consts[b, p, k] = mask bias for key k*128+p
    bias = np.where(attn_mask, NEG, np.float32(0.0)).astype(np.float32)
    consts = np.ascontiguousarray(bias.reshape(B, KT, P).transpose(0, 2, 1))

    ones = np.ones((P, 1), dtype=np.float32)
    return kq, vvl, consts, ones


def kernel(q, k, v, ratio, scale, attn_mask, _want_trace=False):
    """Full inputs in, full output out. Shards batch across 8 cores."""
    assert q.shape == (B, LQ, D) and k.shape == (B, LK, D)
    nc = _get_nc()
    kq, vvl, consts, ones = _pack_inputs(q, k, v, ratio, scale, attn_mask)

    in_maps = [
        {"kq": kq[b], "vv": vvl[b], "consts": consts[b], "onesd": ones}
        for b in range(B)
    ]
    res = run_bass_kernel_spmd(
        nc, in_maps, core_ids=list(range(NCORES)), trace=_want_trace,
    )
    out_un = np.stack([r["out_u"] for r in res.results])         # [B, LQ, D]
    ssum = np.stack([r["sums"] for r in res.results]).reshape(B, LQ)
    out = out_un / ssum[:, :, None]
    if _want_trace:
        return out.astype(np.float32), res
    return out.astype(np.float32)


# revision 10
# speedup vs baseline: 364.8260x; 1.4090x over previous
"""Sparse-attention Bass kernel for Trainium2 (8 NeuronCores).

Problem (per batch element b of 8):
    scores = (q @ k^T) * scale            [2048, 2048]
    scores = where(mask[k], -1e9, scores)
    scores = scores * ratio[b]
    attn   = softmax(scores, axis=-1)
    out    = attn @ v                      [2048, 512]

Sharding: batch dim (8) -> one NeuronCore each (SPMD, same NEFF).

Device layout ("S^T layout"): scores are computed transposed,
S^T[k, q] = K @ Q^T (keys on partitions, queries on the free dim), so
  - the key-mask bias is a per-partition bias -> fused into the exp
    activation on the Scalar engine for free,
  - the AV matmul (contraction over keys) needs no transposes:
    lhsT = P^T tile [128k, 128q] (stationary), rhs = V [128k, 512d],
  - softmax denominators (sum over keys = partitions) come from a
    ones-vector matmul: rowsum[1, q] += ones[128,1].T @ P^T[128, q].

Normalization (divide by rowsum) is done on the host: the device returns
the unnormalized O = exp(S) @ V plus the row sums.
The scale*ratio[b] factor is folded into q on the host.

Written in raw Bass (explicit engine programs + semaphores): the walrus
build in this container allows at most ONE semaphore wait per
instruction, which the Tile scheduler's auto-generated waits violate.
Standalone wait_ge instructions sidestep the limit.

Engine roles:
  sync   (SP) : input DMAs (one HWDGE ring, FIFO -> one dma_sem)
  tensor (PE) : QK^T matmuls, rowsum matmuls, AV matmuls (float32r)
  scalar (ACT): exp (+mask bias), PSUM->SBUF copies, output DMAs
                (on ACT's own HWDGE ring so they don't queue behind
                the input DMAs)
"""

import sys

for _p in ("/opt/trn_rl_repo", "/opt/pypackages"):
    if _p not in sys.path:
        sys.path.append(_p)

import numpy as np
from contextlib import ExitStack

import concourse.bass as bass
from concourse import mybir
from concourse.bass_utils import run_bass_kernel_spmd

B, LQ, LK, D = 8, 2048, 2048, 512
P = 128
NCORES = 8
F32 = mybir.dt.float32
F32R = mybir.dt.float32r
NEG = np.float32(-1e9)

DT = D // P        # 4 d-tiles (contraction for QK^T)
KT = LK // P       # 16 key tiles (partitions of S^T)
QBS = 512          # queries per PSUM block (free dim of S^T)
QB = LQ // QBS     # 4 query superblocks
QTPB = QBS // P    # 4 query tiles (of 128) per superblock

# kq packing: [128, 16384] =
#   A(cols 0:2048)      kT d-tiles, keys 0:512
#   B(cols 2048:4096)   qT d-tiles, queries 0:512
#   C(cols 4096:10240)  kT d-tiles, keys 512:2048
#   D(cols 10240:16384) qT d-tiles, queries 512:2048
KQ_COLS = 4 * (LK + LQ) // P * P  # 16384
C0, D0 = 4096, 10240


def _kcol(d, j):
    """column of kq holding kT[d*128+p, j]"""
    return d * 512 + j if j < 512 else C0 + d * 1536 + (j - 512)


def _qcol(d, i):
    return 2048 + d * 512 + i if i < 512 else D0 + d * 1536 + (i - 512)


def _build_bass(niter=1):
    nc = bass.Bass()

    consts = nc.dram_tensor("consts", [P, KT], F32, kind="ExternalInput")
    onesd = nc.dram_tensor("onesd", [P, 1], F32R, kind="ExternalInput")
    kq = nc.dram_tensor("kq", [P, KQ_COLS], F32R, kind="ExternalInput")
    vv = nc.dram_tensor("vv", [P, KT * D], F32R, kind="ExternalInput")
    out_u = nc.dram_tensor("out_u", [LQ, D], F32, kind="ExternalOutput")
    sums = nc.dram_tensor("sums", [QB, QBS], F32, kind="ExternalOutput")

    EXP = mybir.ActivationFunctionType.Exp

    with ExitStack() as ctx:
        e = ctx.enter_context

        # SBUF
        sb_consts = e(nc.sbuf_tensor("sb_consts", [P, KT], F32))
        sb_ones = e(nc.sbuf_tensor("sb_ones", [P, 1], F32R))
        sb_kq = e(nc.sbuf_tensor("sb_kq", [P, KQ_COLS], F32R))
        sb_v = e(nc.sbuf_tensor("sb_v", [P, KT * D], F32R))
        # exp(S^T) tiles: [128k, 512q] per (qb parity, key tile)
        sb_pt = [
            [e(nc.sbuf_tensor(f"sb_pt{par}_{k}", [P, QBS], F32R)) for k in range(KT)]
            for par in range(2)
        ]
        sb_osb = [e(nc.sbuf_tensor(f"sb_osb{qt}", [P, D], F32)) for qt in range(QTPB)]
        sb_rs = [e(nc.sbuf_tensor(f"sb_rs{par}", [1, QBS], F32)) for par in range(2)]
        # per-partition partial sums of exp tiles (DVE), consumed by one
        # ones-matmul per block on PE
        sb_acc = [e(nc.sbuf_tensor(f"sb_acc{par}", [P, QBS], F32R)) for par in range(2)]

        # PSUM: 7 of 8 banks
        ps = [e(nc.psum_tensor(f"ps{i}", [P, QBS], F32)) for i in range(3)]
        po = [e(nc.psum_tensor(f"po{i}", [P, D], F32)) for i in range(2)]
        rs = [e(nc.psum_tensor(f"rs{i}", [P, QBS], F32)) for i in range(2)]

        # one semaphore per input DMA: HWDGE DMAs on one ring may
        # complete out of order, so a shared counter can't identify which
        # transfer landed
        s_consts = e(nc.semaphore("s_consts"))
        s_ones = e(nc.semaphore("s_ones"))
        s_ab = e(nc.semaphore("s_ab"))
        s_c = e(nc.semaphore("s_c"))
        s_d = e(nc.semaphore("s_d"))
        s_v = e(nc.semaphore("s_v"))
        # per-output-buffer DMA-completion semaphores (buffer reuse gates)
        s_osb = [e(nc.semaphore(f"s_osb{qt}")) for qt in range(QTPB)]
        s_rsb = [e(nc.semaphore(f"s_rsb{par}")) for par in range(2)]
        pe_sem = e(nc.semaphore("pe_sem"))
        act_sem = e(nc.semaphore("act_sem"))
        dve_sem = e(nc.semaphore("dve_sem"))

        # ---- semaphore tick bookkeeping ----
        # gb = global block index (niter * QB blocks total); data block
        # qb = gb % QB.
        # pe_sem increments: per gb: 16 QK-group finals, 1 rowsum final,
        # 4 AV finals = 21
        def tick_qk(gb, k):
            return gb * 21 + k + 1

        def tick_av(gb, qt):
            # PE order per block: 16 QK groups, AV qt=0, rowsum MM, AV qt=1..3
            return gb * 21 + (17 if qt == 0 else 18 + qt)

        def tick_rs(gb):
            return gb * 21 + 18

        def tick_acc(gb):
            # dve_sem: 15 accumulate-adds per block
            return 15 * (gb + 1)

        # act_sem increments: per gb: 16 exps, 1 rs copy, 4 po copies = 21
        def tick_exp(gb, k):
            return gb * 21 + k + 1

        def tick_rsc(gb):
            return gb * 21 + 17

        def tick_poc(gb, qt):
            return gb * 21 + 18 + qt

        with nc.Block() as block:

            @block.sync
            def _(sync):
                sync.dma_start(sb_kq[:, 0:C0], kq[:, 0:C0]).then_inc(s_ab, 16)
                sync.dma_start(sb_consts[:, :], consts[:, :]).then_inc(s_consts, 16)
                sync.dma_start(sb_ones[:, :], onesd[:, :]).then_inc(s_ones, 16)
                sync.dma_start(sb_kq[:, C0:D0], kq[:, C0:D0]).then_inc(s_c, 16)
                sync.dma_start(
                    sb_kq[:, D0:KQ_COLS], kq[:, D0:KQ_COLS]
                ).then_inc(s_d, 16)
                sync.dma_start(sb_v[:, :], vv[:, :]).then_inc(s_v, 16)

            @block.tensor
            def _(tensor):
                last_wait = {}  # sem name -> value already waited for

                def wait(sem, val, name):
                    if val > last_wait.get(name, -1):
                        tensor.wait_ge(sem, val)
                        last_wait[name] = val

                for gb in range(niter * QB):
                    qb = gb % QB
                    # ---- QK^T + rowsum phase ----
                    for k in range(KT):
                        g = gb * KT + k  # global k-iteration index
                        # input availability
                        if k < 4 and qb == 0:
                            wait(s_ab, 16, "ab")
                            wait(s_ones, 16, "ones")
                        elif qb == 0:
                            wait(s_c, 16, "c")
                        else:
                            wait(s_d, 16, "d")
                        # ps[g%3] must have been consumed by exp of g-3
                        if g >= 3:
                            g3 = g - 3
                            wait(act_sem, tick_exp(g3 // KT, g3 % KT), "act")
                        for d in range(DT):
                            mm = tensor.matmul(
                                ps[g % 3][:, :],
                                lhsT=sb_kq[:, _kcol(d, k * P):_kcol(d, k * P) + P],
                                rhs=sb_kq[:, _qcol(d, qb * QBS):_qcol(d, qb * QBS) + QBS],
                                start=(d == 0),
                                stop=(d == DT - 1),
                            )
                            if d == DT - 1:
                                mm.then_inc(pe_sem, 1)

                    # ---- AV phase ----
                    wait(s_v, 16, "v")
                    for qt in range(QTPB):
                        # po[qt%2] consumed by copy of (gb,qt-2) / (gb-1,qt+2)
                        if qt >= 2:
                            wait(act_sem, tick_poc(gb, qt - 2), "act")
                        elif gb >= 1:
                            wait(act_sem, tick_poc(gb - 1, qt + 2), "act")
                        for k in range(KT):
                            if qt == 0:
                                wait(act_sem, tick_exp(gb, k), "act")
                            mm = tensor.matmul(
                                po[qt % 2][:, :],
                                lhsT=sb_pt[gb % 2][k][:, qt * P:(qt + 1) * P],
                                rhs=sb_v[:, k * D:(k + 1) * D],
                                start=(k == 0),
                                stop=(k == KT - 1),
                            )
                            if k == KT - 1:
                                mm.then_inc(pe_sem, 1)
                        if qt == 0:
                            # single partition-reduction matmul over the
                            # DVE-accumulated exp sums
                            wait(dve_sem, tick_acc(gb), "dve")
                            tensor.matmul(
                                rs[gb % 2][0:1, :],
                                lhsT=sb_ones[:, :],
                                rhs=sb_acc[gb % 2][:, :],
                                start=True,
                                stop=True,
                            ).then_inc(pe_sem, 1)

            @block.vector
            def _(vector):
                last_wait = {}

                def wait(sem, val, name):
                    if val > last_wait.get(name, -1):
                        vector.wait_ge(sem, val)
                        last_wait[name] = val

                ndve = 0
                for gb in range(niter * QB):
                    # acc[gb%2] readable again after PE's rowsum MM of gb-2
                    if gb >= 2:
                        wait(pe_sem, tick_rs(gb - 2), "pe")
                    for j in range(1, KT):
                        wait(act_sem, tick_exp(gb, j), "act")
                        if j > 1:
                            # same-engine RAW on acc: wait for own pipe drain
                            wait(dve_sem, ndve, "dve")
                        vector.tensor_add(
                            sb_acc[gb % 2][:, :],
                            sb_pt[gb % 2][0][:, :] if j == 1 else sb_acc[gb % 2][:, :],
                            sb_pt[gb % 2][j][:, :],
                        ).then_inc(dve_sem, 1)
                        ndve += 1

            @block.scalar
            def _(scalar):
                last_wait = {}

                def wait(sem, val, name):
                    if val > last_wait.get(name, -1):
                        scalar.wait_ge(sem, val)
                        last_wait[name] = val

                wait(s_consts, 16, "consts")
                for gb in range(niter * QB):
                    qb = gb % QB
                    for k in range(KT):
                        g = gb * KT + k
                        wait(pe_sem, tick_qk(gb, k), "pe")
                        scalar.activation(
                            sb_pt[gb % 2][k][:, :],
                            ps[g % 3][:, :],
                            EXP,
                            bias=sb_consts[:, k:k + 1],
                            scale=1.0,
                        ).then_inc(act_sem, 1)
                    # rowsum copy + DMA (ACT's own HWDGE ring)
                    if gb >= 2:
                        wait(s_rsb[gb % 2], 16 * (gb // 2), f"rsb{gb % 2}")
                    wait(pe_sem, tick_rs(gb), "pe")
                    scalar.copy(sb_rs[gb % 2][:, :], rs[gb % 2][0:1, :]).then_inc(
                        act_sem, 1
                    )
                    # self-wait: the DMA engine reads sb_rs asynchronously,
                    # so the copy must have fully drained first
                    wait(act_sem, tick_rsc(gb), "act")
                    scalar.dma_start(
                        sums[qb:qb + 1, :], sb_rs[gb % 2][:, :]
                    ).then_inc(s_rsb[gb % 2], 16)
                    # AV output copies + DMAs
                    for qt in range(QTPB):
                        if gb >= 1:
                            wait(s_osb[qt], 16 * gb, f"osb{qt}")
                        wait(pe_sem, tick_av(gb, qt), "pe")
                        scalar.copy(sb_osb[qt][:, :], po[qt % 2][:, :]).then_inc(
                            act_sem, 1
                        )
                        wait(act_sem, tick_poc(gb, qt), "act")
                        row = (qb * QTPB + qt) * P
                        scalar.dma_start(
                            out_u[row:row + P, :], sb_osb[qt][:, :]
                        ).then_inc(s_osb[qt], 16)
                # drain: all output DMAs landed
                for qt in range(QTPB):
                    scalar.wait_ge(s_osb[qt], 16 * QB * niter)
                for par in range(2):
                    scalar.wait_ge(s_rsb[par], 16 * 2 * niter)

    return nc


_NC_CACHE = {}


def _get_nc(niter=1):
    if niter not in _NC_CACHE:
        _NC_CACHE[niter] = _build_bass(niter)
    return _NC_CACHE[niter]


def _pack_inputs(q, k, v, ratio, scale, attn_mask):
    """Host-side packing into the per-core flat layouts."""
    mult = np.float32(scale) * ratio.astype(np.float32)  # [B]
    qs = q.astype(np.float32) * mult[:, None, None]      # [B, LQ, D]

    # kd[b, d, p, j] = k[b, j, d*128+p]
    kd = np.ascontiguousarray(k.astype(np.float32).transpose(0, 2, 1)).reshape(
        B, DT, P, LK
    )
    qd = np.ascontiguousarray(qs.transpose(0, 2, 1)).reshape(B, DT, P, LQ)

    def reg(x, lo, hi):  # [B, DT, P, hi-lo] -> [B, P, DT*(hi-lo)]
        r = x[:, :, :, lo:hi].transpose(0, 2, 1, 3)
        return np.ascontiguousarray(r).reshape(B, P, -1)

    kq = np.concatenate(
        [reg(kd, 0, 512), reg(qd, 0, 512), reg(kd, 512, LK), reg(qd, 512, LQ)],
        axis=2,
    )  # [B, 128, 16384]

    vvl = (
        v.astype(np.float32).reshape(B, KT, P, D).transpose(0, 2, 1, 3)
    )
    vvl = np.ascontiguousarray(vvl).reshape(B, P, KT * D)

    # consts[b, p, k] = mask bias for key k*128+p
    bias = np.where(attn_mask, NEG, np.float32(0.0)).astype(np.float32)
    consts = np.ascontiguousarray(bias.reshape(B, KT, P).transpose(0, 2, 1))

    ones = np.ones((P, 1), dtype=np.float32)
    return kq, vvl, consts, ones


def kernel(q, k, v, ratio, scale, attn_mask, _want_trace=False):
    """Full inputs in, full output out. Shards batch across 8 cores."""
    assert q.shape == (B, LQ, D) and k.shape == (B, LK, D)
    nc = _get_nc()
    kq, vvl, consts, ones = _pack_inputs(q, k, v, ratio, scale, attn_mask)

    in_maps = [
        {"kq": kq[b], "vv": vvl[b], "consts": consts[b], "onesd": ones}
        for b in range(B)
    ]
    res = run_bass_kernel_spmd(
        nc, in_maps, core_ids=list(range(NCORES)), trace=_want_trace,
    )
    out_un = np.stack([r["out_u"] for r in res.results])         # [B, LQ, D]
    ssum = np.stack([r["sums"] for r in res.results]).reshape(B, LQ)
    out = out_un / ssum[:, :, None]
    if _want_trace:
        return out.astype(np.float32), res
    return out.astype(np.float32)
